# revision 16
# baseline (speedup 1.0000x reference)
"""Trainium2 Bass kernel for a 2-layer GCN encoder (GCNConv -> ReLU -> {GCNConv mu, GCNConv logstd}).

Strategy (8 NeuronCores, SPMD):
  - Math: propagate(M) = D^-1/2 (A+I) D^-1/2 M  ==  d * ((A+I) @ (d * M)) with d = deg^-1/2,
    so per-edge norm weights disappear: scale rows by d before and after message passing.
  - Layers 2 and 3 share the propagate: fuse W_mu/W_logstd into one [128,128] matmul + one
    message-passing pass over 128 features, split on the host afterwards.
  - Sharding: nodes are partitioned across the 8 cores (dst-sharding). Each core owns
    N/8 = 6250 output rows and processes the ~E/8 edges pointing into them.
  - Layer-1 linear (x @ W1.T) is sharded: each core computes NPAD/8 table rows, then an
    AllGather (direct to plain DRAM) replicates the table for gathering. Layer-2 likewise.
  - Message passing: dma_gather (HW gather, int16 indices) pulls PAIR rows (2 nodes, 512B)
    from the HBM table viewed as [NPAD/2, 256]; idx = src>>1 < 32767 so no table split.
    Edges are bucketed by (dst tile, src parity) so each 128-edge chunk uses one parity
    half of the gathered rows as matmul lhsT. A one-hot selection matrix (DVE is_equal vs
    iota) turns the segment-sum into PE matmuls accumulated in PSUM per 128-dst-node tile.
  - Gathers are issued round-robin on 4 SWDGE queues: descriptor generation for different
    queues runs concurrently on different Q7 core groups (~3x faster than one queue).
  - All cores run the same program (SPMD): per-(tile,parity) group sizes padded to the
    max over cores.

kernel(**inputs) takes the full-size inputs and returns (mu, logstd) as float32 numpy arrays.
"""
import sys

sys.path.insert(0, "/opt/trn_rl_repo")

import numpy as np
import ml_dtypes

import concourse.bass as bass
import concourse.bacc as bacc
import concourse.mybir as mybir
import concourse.tile as tile
from concourse.bass_utils import run_bass_kernel_spmd

BF16 = ml_dtypes.bfloat16

# ---------------- configuration ----------------
FULL_CFG = dict(
    n=50000,        # nodes
    fin=512,        # input features
    hid=128,        # hidden features
    out2=128,       # fused mu+logstd features
    n_cores=8,
    npad=53248,     # padded node count (multiple of 512*8; NSH=6656=13*512 per core)
    blk=512,        # phase-A block width
    g_edges=8192,   # gather super-chunk (edges per dma_gather)
    swdge_queues=4,
    gather_bufs=4,
    queue_rr=(0, 1, 2, 3),  # round-robin queue assignment for gathers
    ag_shared=False,        # AllGather to Shared scratchpad + bounce (fallback)
)


def _ceil(a, b):
    return -(-a // b)


def preprocess(cfg, x, edge_index, W1, b1, W_mu, b_mu, W_logstd, b_logstd):
    """Host-side: degrees, edge bucketing/padding, operand staging. Returns
    (meta, in_maps). Pure index/layout work plus parameter reformatting."""
    N, C = cfg["n"], cfg["n_cores"]
    NPC = N // C
    T = _ceil(NPC, 128)
    NPAD = cfg["npad"]
    NSH = NPAD // C

    x = np.asarray(x, np.float32)
    ei = np.asarray(edge_index).astype(np.int64)
    W1 = np.asarray(W1, np.float32)
    b1 = np.asarray(b1, np.float32)
    Wcat = np.concatenate([np.asarray(W_mu, np.float32), np.asarray(W_logstd, np.float32)], axis=0)
    bcat = np.concatenate([np.asarray(b_mu, np.float32), np.asarray(b_logstd, np.float32)], axis=0)

    src = np.concatenate([ei[0], np.arange(N, dtype=np.int64)])
    dst = np.concatenate([ei[1], np.arange(N, dtype=np.int64)])
    deg = np.bincount(dst, minlength=N).astype(np.float32)
    dvec = (1.0 / np.sqrt(deg)).astype(np.float32)

    core = dst // NPC
    tloc = (dst % NPC) // 128
    par = (src & 1).astype(np.int64)
    key = (core * T + tloc) * 2 + par
    order = np.argsort(key, kind="stable")
    ss, ds = src[order], dst[order]
    counts = np.bincount(key, minlength=C * T * 2).reshape(C, T, 2)
    gpad = ((counts.max(axis=0) + 127) // 128) * 128  # [T, 2] padded group sizes
    L = int(gpad.sum())
    K_tot = L // 128
    offs = np.concatenate([[0], np.cumsum(counts.reshape(-1))])

    # phase-A staging (per-core x slice)
    xt_full = np.zeros((cfg["fin"], NPAD), BF16)
    xt_full[:, :N] = x.T
    w1t = np.ascontiguousarray(W1.T).astype(BF16)          # [fin, hid]
    wcatt = np.ascontiguousarray(Wcat.T).astype(BF16)      # [hid, out2]
    d_all = np.ones(NPAD, np.float32)
    d_all[:N] = dvec
    iota_arr = np.tile(np.arange(128), (128, 1)).astype(BF16)
    ident = np.eye(128, dtype=BF16)

    in_maps = []
    for c in range(C):
        idxbuf = np.zeros(L, np.int16)
        dstloc_all = np.full(L, 200, np.int32)
        pos = 0
        for t in range(T):
            for h in (0, 1):
                g = int(counts[c, t, h])
                o = int(offs[(c * T + t) * 2 + h])
                sl = slice(o, o + g)
                idxbuf[pos:pos + g] = (ss[sl] >> 1).astype(np.int16)
                dstloc_all[pos:pos + g] = (ds[sl] % NPC) - t * 128
                pos += int(gpad[t, h])
        idx_w = np.tile(idxbuf.reshape(-1, 16).T, (8, 1)).copy()
        dstloc_arr = np.ascontiguousarray(dstloc_all.reshape(-1, 128).T).astype(BF16)

        d_own = dvec[c * NPC:(c + 1) * NPC]
        d_own_pad = np.ones(T * 128, np.float32)
        d_own_pad[:NPC] = d_own
        d_rep = np.tile(d_own_pad, (128, 1)).astype(np.float32)          # [128, T*128]
        d_own_col = np.ascontiguousarray(d_own_pad.reshape(-1, 128).T)   # [128, T]

        d_sh = d_all[c * NSH:(c + 1) * NSH]
        d_sh_col = np.ascontiguousarray(d_sh.reshape(-1, 128).T)         # [128, NSH/128]

        in_maps.append({
            "xt": np.ascontiguousarray(xt_full[:, c * NSH:(c + 1) * NSH]),
            "w1t": w1t, "wcatt": wcatt,
            "b1c": b1.reshape(-1, 1).copy(), "bcatc": bcat.reshape(-1, 1).copy(),
            "dshc": d_sh_col, "drep": d_rep, "downc": d_own_col,
            "iota": iota_arr, "ident": ident,
            "idx": idx_w, "dstloc": dstloc_arr,
        })

    gpt = [[int(gpad[t, 0]) // 128, int(gpad[t, 1]) // 128] for t in range(T)]
    meta = dict(gpt=gpt, L=L, K_tot=K_tot)
    return meta, in_maps


def build_program(cfg, meta):
    N, C = cfg["n"], cfg["n_cores"]
    NPC = N // C
    T = _ceil(NPC, 128)
    NPAD = cfg["npad"]
    NSH = NPAD // C
    FIN, HID, O2 = cfg["fin"], cfg["hid"], cfg["out2"]
    BLK, G = cfg["blk"], cfg["g_edges"]
    KC = FIN // 128
    SPC = G // 128
    gpt, L, K_tot = meta["gpt"], meta["L"], meta["K_tot"]
    bf16 = mybir.dt.bfloat16
    f32 = mybir.dt.float32
    AF = mybir.ActivationFunctionType
    OP = mybir.AluOpType

    nc = bacc.Bacc("TRN2", target_bir_lowering=False, debug=False, num_devices=C,
                   num_swdge_queues=cfg["swdge_queues"])

    xt_d = nc.dram_tensor("xt", [FIN, NSH], bf16, kind="ExternalInput")
    w1t_d = nc.dram_tensor("w1t", [FIN, HID], bf16, kind="ExternalInput")
    wcatt_d = nc.dram_tensor("wcatt", [HID, O2], bf16, kind="ExternalInput")
    b1c_d = nc.dram_tensor("b1c", [HID, 1], f32, kind="ExternalInput")
    bcatc_d = nc.dram_tensor("bcatc", [O2, 1], f32, kind="ExternalInput")
    dshc_d = nc.dram_tensor("dshc", [128, NSH // 128], f32, kind="ExternalInput")
    drep_d = nc.dram_tensor("drep", [128, T * 128], f32, kind="ExternalInput")
    downc_d = nc.dram_tensor("downc", [128, T], f32, kind="ExternalInput")
    iota_d = nc.dram_tensor("iota", [128, 128], bf16, kind="ExternalInput")
    ident_d = nc.dram_tensor("ident", [128, 128], bf16, kind="ExternalInput")
    idx_d = nc.dram_tensor("idx", [128, L // 16], mybir.dt.int16, kind="ExternalInput")
    dstloc_d = nc.dram_tensor("dstloc", [128, K_tot], bf16, kind="ExternalInput")

    g1s_d = nc.dram_tensor("g1s", [NSH, HID], bf16)
    g2s_d = nc.dram_tensor("g2s", [NPC, HID], bf16)
    if cfg["ag_shared"]:
        g1f_sh = nc.dram_tensor("g1fsh", [NPAD, HID], bf16, addr_space="Shared")
        g2f_sh = nc.dram_tensor("g2fsh", [N, HID], bf16, addr_space="Shared")
    g1f_d = nc.dram_tensor("g1f", [NPAD, HID], bf16)
    g2f_d = nc.dram_tensor("g2f", [N, HID], bf16)
    outt_d = nc.dram_tensor("outt", [O2, T * 128], f32, kind="ExternalOutput")

    def pair_view(dram_t, n_pairs):
        ap = dram_t[:, :]
        return bass.AP(ap.tensor, 0, [[256, n_pairs], [1, 256]])

    with tile.TileContext(nc, trace_sim=bool(cfg.get("trace_sim"))) as tc:
        _emit(nc, tc, cfg, meta, locals())
    nc.compile()
    return nc


def _emit(nc, tc, cfg, meta, env):
    N, C = cfg["n"], cfg["n_cores"]
    NPC = N // C
    T = _ceil(NPC, 128)
    NPAD = cfg["npad"]
    NSH = NPAD // C
    FIN, HID, O2 = cfg["fin"], cfg["hid"], cfg["out2"]
    BLK, G = cfg["blk"], cfg["g_edges"]
    KC = FIN // 128
    SPC = G // 128
    gpt, L, K_tot = meta["gpt"], meta["L"], meta["K_tot"]
    bf16 = mybir.dt.bfloat16
    f32 = mybir.dt.float32
    AF = mybir.ActivationFunctionType
    OP = mybir.AluOpType
    xt_d, w1t_d, wcatt_d = env["xt_d"], env["w1t_d"], env["wcatt_d"]
    b1c_d, bcatc_d = env["b1c_d"], env["bcatc_d"]
    dshc_d, drep_d, downc_d = env["dshc_d"], env["drep_d"], env["downc_d"]
    iota_d, ident_d, idx_d, dstloc_d = env["iota_d"], env["ident_d"], env["idx_d"], env["dstloc_d"]
    g1s_d, g2s_d, g1f_d, g2f_d, outt_d = (env["g1s_d"], env["g2s_d"], env["g1f_d"],
                                          env["g2f_d"], env["outt_d"])
    g1f_sh = env.get("g1f_sh")
    g2f_sh = env.get("g2f_sh")
    pair_view = env["pair_view"]

    with tc.tile_pool(name="const", bufs=1) as const_p:
            w1t_sb = []
            for kc in range(KC):
                w = const_p.tile([128, HID], bf16, tag=f"w1t{kc}")
                nc.sync.dma_start(w[:], w1t_d[kc * 128:(kc + 1) * 128, :])
                w1t_sb.append(w)
            wcatt_sb = const_p.tile([HID, O2], bf16, tag="wcatt")
            nc.sync.dma_start(wcatt_sb[:], wcatt_d[:])
            b1_sb = const_p.tile([HID, 1], f32, tag="b1")
            nc.sync.dma_start(b1_sb[:], b1c_d[:])
            bcat_sb = const_p.tile([O2, 1], f32, tag="bcat")
            nc.sync.dma_start(bcat_sb[:], bcatc_d[:])
            dshc_sb = const_p.tile([128, NSH // 128], f32, tag="dshc")
            nc.sync.dma_start(dshc_sb[:], dshc_d[:])
            drep_sb = const_p.tile([128, T * 128], f32, tag="drep")
            nc.sync.dma_start(drep_sb[:], drep_d[:])
            downc_sb = const_p.tile([128, T], f32, tag="downc")
            nc.sync.dma_start(downc_sb[:], downc_d[:])
            iota_sb = const_p.tile([128, 128], bf16, tag="iota")
            nc.sync.dma_start(iota_sb[:], iota_d[:])
            ident_sb = const_p.tile([128, 128], bf16, tag="ident")
            nc.sync.dma_start(ident_sb[:], ident_d[:])
            idx_sb = const_p.tile([128, L // 16], mybir.dt.int16, tag="idx")
            nc.sync.dma_start(idx_sb[:], idx_d[:])
            dstloc_sb = const_p.tile([128, K_tot], bf16, tag="dstloc")
            nc.sync.dma_start(dstloc_sb[:], dstloc_d[:])

            # ---------------- phase A: g1 shard = d * (x_shard @ W1.T)
            scA, _ = nc.enter_named_scope("phaseA", False)
            with tc.tile_pool(name="pa_x", bufs=2) as xt_p, \
                 tc.tile_pool(name="pa_t", bufs=3) as t1_p, \
                 tc.tile_pool(name="pa_w", bufs=3) as wst_p, \
                 tc.tile_pool(name="pa_ps", bufs=2, space="PSUM") as pa, \
                 tc.tile_pool(name="pa_ps2", bufs=2, space="PSUM") as pb:
                for blki in range(NSH // BLK):
                    xts = []
                    for kc in range(KC):
                        xk = xt_p.tile([128, BLK], bf16, tag=f"xt{kc}")
                        nc.sync.dma_start(xk[:], xt_d[kc * 128:(kc + 1) * 128,
                                                      blki * BLK:(blki + 1) * BLK])
                        xts.append(xk)
                    ps_a = pa.tile([128, BLK], f32, space="PSUM", tag="psa")
                    for kc in range(KC):
                        nc.tensor.matmul(ps_a[:], lhsT=w1t_sb[kc][:], rhs=xts[kc][:],
                                         start=(kc == 0), stop=(kc == KC - 1))
                    t1t = t1_p.tile([128, BLK], bf16, tag="t1t")
                    nc.scalar.copy(t1t[:], ps_a[:])
                    sb = BLK // 128
                    ps_b = pb.tile([128, sb, 128], bf16, space="PSUM", tag="psb")
                    for s in range(sb):
                        nc.tensor.transpose(ps_b[:, s, :], t1t[:, s * 128:(s + 1) * 128],
                                            ident_sb[:])
                    wst = wst_p.tile([128, sb, HID], bf16, tag="wst")
                    # wst[p, s, f] = ps_b[p, s, f] * d[blk0 + s*128 + p]
                    nb0 = blki * sb
                    dsl = dshc_sb[:, nb0:nb0 + sb]
                    in1 = bass.AP(dsl.tensor, dsl.offset,
                                  [dsl.ap[0], [dsl.ap[1][0], sb], [0, 128]])
                    nc.vector.tensor_tensor(out=wst[:], in0=ps_b[:], in1=in1, op=OP.mult)
                    r0 = blki * BLK
                    nc.sync.dma_start(
                        g1s_d[r0:r0 + BLK, :].rearrange("(s p) f -> p s f", p=128), wst[:])
            # AllGather the shard table
            if cfg["ag_shared"]:
                nc.gpsimd.collective_compute(
                    "AllGather", OP.bypass, replica_groups=[list(range(C))],
                    ins=[g1s_d[:]], outs=[g1f_sh[:]])
                nc.sync.dma_start(g1f_d[:, :], g1f_sh[:, :])
            else:
                nc.gpsimd.collective_compute(
                    "AllGather", OP.bypass, replica_groups=[list(range(C))],
                    ins=[g1s_d[:]], outs=[g1f_d[:]])
            nc.leave_named_scope("phaseA", scA, False)
            if cfg.get("stop_after") == "A":
                _drain_out(nc, tc, outt_d)
                return

            # ---------------- message passing (both layers)
            def propagate(table_pairs, finalize):
                qrr = cfg["queue_rr"]
                with tc.tile_pool(name="mp_g", bufs=cfg["gather_bufs"]) as gath_p, \
                     tc.tile_pool(name="mp_oh", bufs=3) as oh_p, \
                     tc.tile_pool(name="mp_ps", bufs=4, space="PSUM") as psp:
                    gh = []
                    for i in range(_ceil(L, G)):
                        n_i = min(G, L - i * G)
                        gt = gath_p.tile([128, SPC, 256], bf16, tag="gt", name="gt")
                        nc.gpsimd.dma_gather(
                            out_ap=gt[:, :n_i // 128, :],
                            in_ap=table_pairs,
                            idxs_ap=idx_sb[:, i * (G // 16):i * (G // 16) + n_i // 16],
                            num_idxs=n_i,
                            num_idxs_reg=n_i,
                            elem_size=256,
                            single_packet=False,
                            queue_num=qrr[i % len(qrr)],
                        )
                        gh.append(gt)
                    kk = 0  # global chunk counter
                    kg = 0  # dstloc column counter
                    for t in range(T):
                        nch_t = gpt[t][0] + gpt[t][1]
                        ps_t = psp.tile([128, 128], f32, space="PSUM", tag="ps", name="ps_t")
                        j = 0
                        for h in (0, 1):
                            nch = gpt[t][h]
                            if nch == 0:
                                continue
                            oh = oh_p.tile([128, nch, 128], bf16, tag="oh", name="oh")
                            dsl = dstloc_sb[:, kg:kg + nch]
                            in0 = bass.AP(dsl.tensor, dsl.offset,
                                          [dsl.ap[0], [dsl.ap[1][0], nch], [0, 128]])
                            io = iota_sb[:]
                            in1 = bass.AP(io.tensor, io.offset,
                                          [io.ap[0], [0, nch], io.ap[1]])
                            nc.vector.tensor_tensor(out=oh[:], in0=in0, in1=in1,
                                                    op=OP.is_equal)
                            for jj in range(nch):
                                gt = gh[kk // SPC]
                                nc.tensor.matmul(
                                    ps_t[:],
                                    lhsT=gt[:, kk % SPC, h * 128:(h + 1) * 128],
                                    rhs=oh[:, jj, :],
                                    start=(j == 0), stop=(j == nch_t - 1))
                                kk += 1
                                j += 1
                            kg += nch
                        if nch_t == 0:
                            nc.vector.memset(ps_t[:], 0.0)
                        finalize(t, ps_t)

            with tc.tile_pool(name="ht", bufs=1) as ht_p, \
                 tc.tile_pool(name="fin", bufs=4) as fin_p:
                ht = {}

                def fin1(t, acc_t):
                    tmp = fin_p.tile([128, 128], f32, tag="tmp")
                    nc.vector.tensor_tensor(out=tmp[:], in0=acc_t[:],
                                            in1=drep_sb[:, t * 128:(t + 1) * 128],
                                            op=OP.mult)
                    h_t = ht_p.tile([128, 128], bf16, tag=f"ht{t}")
                    nc.scalar.activation(h_t[:], tmp[:], AF.Relu, bias=b1_sb[:])
                    ht[t] = h_t

                scP1, _ = nc.enter_named_scope("prop1", False)
                propagate(pair_view(g1f_d, NPAD // 2), fin1)
                nc.leave_named_scope("prop1", scP1, False)
                if cfg.get("stop_after") == "P1":
                    _drain_out(nc, tc, outt_d)
                    return

                # ---------------- phase C: g2 shard = d * (h @ Wcat.T), AllGather
                scC, _ = nc.enter_named_scope("phaseC", False)
                with tc.tile_pool(name="pc_t", bufs=4) as ct_p, \
                     tc.tile_pool(name="pc_ps", bufs=2, space="PSUM") as pc1, \
                     tc.tile_pool(name="pc_ps2", bufs=2, space="PSUM") as pc2:
                    for t in range(T):
                        ps = pc1.tile([O2, 128], f32, space="PSUM", tag="c1")
                        nc.tensor.matmul(ps[:], lhsT=wcatt_sb[:], rhs=ht[t][:],
                                         start=True, stop=True)
                        c_sb = ct_p.tile([O2, 128], bf16, tag="csb")
                        nc.scalar.copy(c_sb[:], ps[:])
                        ps2 = pc2.tile([128, O2], bf16, space="PSUM", tag="c2")
                        nc.tensor.transpose(ps2[:], c_sb[:], ident_sb[:])
                        g2t = ct_p.tile([128, O2], bf16, tag="g2t")
                        nc.vector.tensor_scalar_mul(g2t[:], ps2[:], downc_sb[:, t:t + 1])
                        nrows = min(128, NPC - t * 128)
                        nc.sync.dma_start(g2s_d[t * 128:t * 128 + nrows, :], g2t[:nrows, :])
                if cfg["ag_shared"]:
                    nc.gpsimd.collective_compute(
                        "AllGather", OP.bypass, replica_groups=[list(range(C))],
                        ins=[g2s_d[:]], outs=[g2f_sh[:]])
                    nc.sync.dma_start(g2f_d[:, :], g2f_sh[:, :])
                else:
                    nc.gpsimd.collective_compute(
                        "AllGather", OP.bypass, replica_groups=[list(range(C))],
                        ins=[g2s_d[:]], outs=[g2f_d[:]])
                nc.leave_named_scope("phaseC", scC, False)

                # ---------------- phase D: second propagate + output
                def fin2(t, acc_t):
                    tmp = fin_p.tile([128, 128], f32, tag="tmp")
                    nc.vector.tensor_tensor(out=tmp[:], in0=acc_t[:],
                                            in1=drep_sb[:, t * 128:(t + 1) * 128],
                                            op=OP.mult)
                    osb = fin_p.tile([O2, 128], f32, tag="osb")
                    nc.scalar.activation(osb[:], tmp[:], AF.Identity, bias=bcat_sb[:])
                    nc.sync.dma_start(outt_d[:, t * 128:(t + 1) * 128], osb[:])

                scP2, _ = nc.enter_named_scope("prop2", False)
                propagate(pair_view(g2f_d, N // 2), fin2)
                nc.leave_named_scope("prop2", scP2, False)


def _drain_out(nc, tc, outt_d):
    """Make truncated (stop_after) programs still produce the output tensor."""
    with tc.tile_pool(name="drain", bufs=1) as dp:
        z = dp.tile([128, 16], mybir.dt.float32, tag="z")
        nc.vector.memset(z[:], 0.0)
        nc.sync.dma_start(outt_d[:, 0:16], z[:])


def run(cfg, x, edge_index, W1, b1, W_mu, b_mu, W_logstd, b_logstd, program_cache=None,
        trace=False, result_box=None):
    meta, in_maps = preprocess(cfg, x, edge_index, W1, b1, W_mu, b_mu, W_logstd, b_logstd)
    nc = build_program(cfg, meta)
    res = run_bass_kernel_spmd(nc, in_maps, list(range(cfg["n_cores"])), trace=trace)
    if result_box is not None:
        result_box.append(res)
    N, C = cfg["n"], cfg["n_cores"]
    NPC = N // C
    O = cfg["out2"] // 2
    mu = np.empty((N, O), np.float32)
    logstd = np.empty((N, O), np.float32)
    for c in range(C):
        ot = res.results[c]["outt"]
        mu[c * NPC:(c + 1) * NPC] = ot[:O, :NPC].T
        logstd[c * NPC:(c + 1) * NPC] = ot[O:, :NPC].T
    return mu, logstd


def kernel(x, edge_index, W1, b1, W_mu, b_mu, W_logstd, b_logstd):
    mu, logstd = run(FULL_CFG, x, edge_index, W1, b1, W_mu, b_mu, W_logstd, b_logstd)
    return mu, logstd


# revision 17
# speedup vs baseline: 1.2078x; 1.2078x over previous
"""Trainium2 Bass kernel for a 2-layer GCN encoder (GCNConv -> ReLU -> {GCNConv mu, GCNConv logstd}).

Strategy (8 NeuronCores, SPMD):
  - Math: propagate(M) = D^-1/2 (A+I) D^-1/2 M  ==  d * ((A+I) @ (d * M)) with d = deg^-1/2,
    so per-edge norm weights disappear: scale rows by d before and after message passing.
  - Layers 2 and 3 share the propagate: fuse W_mu/W_logstd into one [128,128] matmul + one
    message-passing pass over 128 features, split on the host afterwards.
  - Sharding: nodes are partitioned across the 8 cores (dst-sharding). Each core owns
    N/8 = 6250 output rows and processes the ~E/8 edges pointing into them.
  - Layer-1 linear (x @ W1.T) is sharded: each core computes NPAD/8 table rows, then an
    AllGather (direct to plain DRAM) replicates the table for gathering. Layer-2 likewise.
  - Message passing: dma_gather (HW gather, int16 indices) pulls PAIR rows (2 nodes, 512B)
    from the HBM table viewed as [NPAD/2, 256]; idx = src>>1 < 32767 so no table split.
    Edges are bucketed by (dst tile, src parity) so each 128-edge chunk uses one parity
    half of the gathered rows as matmul lhsT. A one-hot selection matrix (DVE is_equal vs
    iota) turns the segment-sum into PE matmuls accumulated in PSUM per 128-dst-node tile.
  - Gathers are issued round-robin on 4 SWDGE queues: descriptor generation for different
    queues runs concurrently on different Q7 core groups (~3x faster than one queue).
  - All cores run the same program (SPMD): per-(tile,parity) group sizes padded to the
    max over cores.

kernel(**inputs) takes the full-size inputs and returns (mu, logstd) as float32 numpy arrays.
"""
import sys

sys.path.insert(0, "/opt/trn_rl_repo")

import numpy as np
import ml_dtypes

import concourse.bass as bass
import concourse.bacc as bacc
import concourse.mybir as mybir
import concourse.tile as tile
from concourse.bass_utils import run_bass_kernel_spmd

BF16 = ml_dtypes.bfloat16

# ---------------- configuration ----------------
FULL_CFG = dict(
    n=50000,        # nodes
    fin=512,        # input features
    hid=128,        # hidden features
    out2=128,       # fused mu+logstd features
    n_cores=8,
    npad=53248,     # padded node count (multiple of 512*8; NSH=6656=13*512 per core)
    blk=512,        # phase-A block width
    g_edges=4096,   # gather super-chunk (edges per dma_gather)
    swdge_queues=4,
    gather_bufs=7,
    queue_rr=(0, 1, 2, 3),  # round-robin queue assignment for gathers
    ag_shared=False,        # AllGather to Shared scratchpad + bounce (fallback)
)


def _ceil(a, b):
    return -(-a // b)


def preprocess(cfg, x, edge_index, W1, b1, W_mu, b_mu, W_logstd, b_logstd):
    """Host-side: degrees, edge bucketing/padding, operand staging. Returns
    (meta, in_maps). Pure index/layout work plus parameter reformatting."""
    N, C = cfg["n"], cfg["n_cores"]
    NPC = N // C
    T = _ceil(NPC, 128)
    NPAD = cfg["npad"]
    NSH = NPAD // C

    x = np.asarray(x, np.float32)
    ei = np.asarray(edge_index).astype(np.int64)
    W1 = np.asarray(W1, np.float32)
    b1 = np.asarray(b1, np.float32)
    Wcat = np.concatenate([np.asarray(W_mu, np.float32), np.asarray(W_logstd, np.float32)], axis=0)
    bcat = np.concatenate([np.asarray(b_mu, np.float32), np.asarray(b_logstd, np.float32)], axis=0)

    src = np.concatenate([ei[0], np.arange(N, dtype=np.int64)])
    dst = np.concatenate([ei[1], np.arange(N, dtype=np.int64)])
    deg = np.bincount(dst, minlength=N).astype(np.float32)
    dvec = (1.0 / np.sqrt(deg)).astype(np.float32)

    core = dst // NPC
    tloc = (dst % NPC) // 128
    par = (src & 1).astype(np.int64)
    key = (core * T + tloc) * 2 + par
    order = np.argsort(key, kind="stable")
    ss, ds = src[order], dst[order]
    counts = np.bincount(key, minlength=C * T * 2).reshape(C, T, 2)
    gpad = ((counts.max(axis=0) + 127) // 128) * 128  # [T, 2] padded group sizes
    L = int(gpad.sum())
    K_tot = L // 128
    offs = np.concatenate([[0], np.cumsum(counts.reshape(-1))])

    # phase-A staging (per-core x slice)
    xt_full = np.zeros((cfg["fin"], NPAD), BF16)
    xt_full[:, :N] = x.T
    w1t = np.ascontiguousarray(W1.T).astype(BF16)          # [fin, hid]
    wcatt = np.ascontiguousarray(Wcat.T).astype(BF16)      # [hid, out2]
    d_all = np.ones(NPAD, np.float32)
    d_all[:N] = dvec
    iota_arr = np.tile(np.arange(128), (128, 1)).astype(BF16)
    ident = np.eye(128, dtype=BF16)

    in_maps = []
    for c in range(C):
        idxbuf = np.zeros(L, np.int16)
        dstloc_all = np.full(L, 200, np.int32)
        pos = 0
        for t in range(T):
            for h in (0, 1):
                g = int(counts[c, t, h])
                o = int(offs[(c * T + t) * 2 + h])
                sl = slice(o, o + g)
                idxbuf[pos:pos + g] = (ss[sl] >> 1).astype(np.int16)
                dstloc_all[pos:pos + g] = (ds[sl] % NPC) - t * 128
                pos += int(gpad[t, h])
        idx_w = np.tile(idxbuf.reshape(-1, 16).T, (8, 1)).copy()
        dstloc_arr = np.ascontiguousarray(dstloc_all.reshape(-1, 128).T).astype(BF16)

        d_own = dvec[c * NPC:(c + 1) * NPC]
        d_own_pad = np.ones(T * 128, np.float32)
        d_own_pad[:NPC] = d_own
        d_rep = np.tile(d_own_pad, (128, 1)).astype(np.float32)          # [128, T*128]
        d_own_col = np.ascontiguousarray(d_own_pad.reshape(-1, 128).T)   # [128, T]

        d_sh = d_all[c * NSH:(c + 1) * NSH]
        d_sh_col = np.ascontiguousarray(d_sh.reshape(-1, 128).T)         # [128, NSH/128]

        in_maps.append({
            "xt": np.ascontiguousarray(xt_full[:, c * NSH:(c + 1) * NSH]),
            "w1t": w1t, "wcatt": wcatt,
            "b1c": b1.reshape(-1, 1).copy(), "bcatc": bcat.reshape(-1, 1).copy(),
            "dshc": d_sh_col, "drep": d_rep, "downc": d_own_col,
            "iota": iota_arr, "ident": ident,
            "idx": idx_w, "dstloc": dstloc_arr,
        })

    gpt = [[int(gpad[t, 0]) // 128, int(gpad[t, 1]) // 128] for t in range(T)]
    meta = dict(gpt=gpt, L=L, K_tot=K_tot)
    return meta, in_maps


def build_program(cfg, meta):
    N, C = cfg["n"], cfg["n_cores"]
    NPC = N // C
    T = _ceil(NPC, 128)
    NPAD = cfg["npad"]
    NSH = NPAD // C
    FIN, HID, O2 = cfg["fin"], cfg["hid"], cfg["out2"]
    BLK, G = cfg["blk"], cfg["g_edges"]
    KC = FIN // 128
    SPC = G // 128
    gpt, L, K_tot = meta["gpt"], meta["L"], meta["K_tot"]
    bf16 = mybir.dt.bfloat16
    f32 = mybir.dt.float32
    AF = mybir.ActivationFunctionType
    OP = mybir.AluOpType

    nc = bacc.Bacc("TRN2", target_bir_lowering=False, debug=False, num_devices=C,
                   num_swdge_queues=cfg["swdge_queues"])

    xt_d = nc.dram_tensor("xt", [FIN, NSH], bf16, kind="ExternalInput")
    w1t_d = nc.dram_tensor("w1t", [FIN, HID], bf16, kind="ExternalInput")
    wcatt_d = nc.dram_tensor("wcatt", [HID, O2], bf16, kind="ExternalInput")
    b1c_d = nc.dram_tensor("b1c", [HID, 1], f32, kind="ExternalInput")
    bcatc_d = nc.dram_tensor("bcatc", [O2, 1], f32, kind="ExternalInput")
    dshc_d = nc.dram_tensor("dshc", [128, NSH // 128], f32, kind="ExternalInput")
    drep_d = nc.dram_tensor("drep", [128, T * 128], f32, kind="ExternalInput")
    downc_d = nc.dram_tensor("downc", [128, T], f32, kind="ExternalInput")
    iota_d = nc.dram_tensor("iota", [128, 128], bf16, kind="ExternalInput")
    ident_d = nc.dram_tensor("ident", [128, 128], bf16, kind="ExternalInput")
    idx_d = nc.dram_tensor("idx", [128, L // 16], mybir.dt.int16, kind="ExternalInput")
    dstloc_d = nc.dram_tensor("dstloc", [128, K_tot], bf16, kind="ExternalInput")

    g1s_d = nc.dram_tensor("g1s", [NSH, HID], bf16)
    g2s_d = nc.dram_tensor("g2s", [NPC, HID], bf16)
    if cfg["ag_shared"]:
        g1f_sh = nc.dram_tensor("g1fsh", [NPAD, HID], bf16, addr_space="Shared")
        g2f_sh = nc.dram_tensor("g2fsh", [N, HID], bf16, addr_space="Shared")
    g1f_d = nc.dram_tensor("g1f", [NPAD, HID], bf16)
    g2f_d = nc.dram_tensor("g2f", [N, HID], bf16)
    outt_d = nc.dram_tensor("outt", [O2, T * 128], f32, kind="ExternalOutput")

    def pair_view(dram_t, n_pairs):
        ap = dram_t[:, :]
        return bass.AP(ap.tensor, 0, [[256, n_pairs], [1, 256]])

    with tile.TileContext(nc, trace_sim=bool(cfg.get("trace_sim"))) as tc:
        _emit(nc, tc, cfg, meta, locals())
    nc.compile()
    return nc


def _emit(nc, tc, cfg, meta, env):
    N, C = cfg["n"], cfg["n_cores"]
    NPC = N // C
    T = _ceil(NPC, 128)
    NPAD = cfg["npad"]
    NSH = NPAD // C
    FIN, HID, O2 = cfg["fin"], cfg["hid"], cfg["out2"]
    BLK, G = cfg["blk"], cfg["g_edges"]
    KC = FIN // 128
    SPC = G // 128
    gpt, L, K_tot = meta["gpt"], meta["L"], meta["K_tot"]
    bf16 = mybir.dt.bfloat16
    f32 = mybir.dt.float32
    AF = mybir.ActivationFunctionType
    OP = mybir.AluOpType
    xt_d, w1t_d, wcatt_d = env["xt_d"], env["w1t_d"], env["wcatt_d"]
    b1c_d, bcatc_d = env["b1c_d"], env["bcatc_d"]
    dshc_d, drep_d, downc_d = env["dshc_d"], env["drep_d"], env["downc_d"]
    iota_d, ident_d, idx_d, dstloc_d = env["iota_d"], env["ident_d"], env["idx_d"], env["dstloc_d"]
    g1s_d, g2s_d, g1f_d, g2f_d, outt_d = (env["g1s_d"], env["g2s_d"], env["g1f_d"],
                                          env["g2f_d"], env["outt_d"])
    g1f_sh = env.get("g1f_sh")
    g2f_sh = env.get("g2f_sh")
    pair_view = env["pair_view"]

    with tc.tile_pool(name="const", bufs=1) as const_p:
            w1t_sb = []
            for kc in range(KC):
                w = const_p.tile([128, HID], bf16, tag=f"w1t{kc}")
                nc.sync.dma_start(w[:], w1t_d[kc * 128:(kc + 1) * 128, :])
                w1t_sb.append(w)
            wcatt_sb = const_p.tile([HID, O2], bf16, tag="wcatt")
            nc.sync.dma_start(wcatt_sb[:], wcatt_d[:])
            b1_sb = const_p.tile([HID, 1], f32, tag="b1")
            nc.sync.dma_start(b1_sb[:], b1c_d[:])
            bcat_sb = const_p.tile([O2, 1], f32, tag="bcat")
            nc.sync.dma_start(bcat_sb[:], bcatc_d[:])
            dshc_sb = const_p.tile([128, NSH // 128], f32, tag="dshc")
            nc.sync.dma_start(dshc_sb[:], dshc_d[:])
            drep_sb = const_p.tile([128, T * 128], f32, tag="drep")
            nc.sync.dma_start(drep_sb[:], drep_d[:])
            downc_sb = const_p.tile([128, T], f32, tag="downc")
            nc.sync.dma_start(downc_sb[:], downc_d[:])
            iota_sb = const_p.tile([128, 128], bf16, tag="iota")
            nc.sync.dma_start(iota_sb[:], iota_d[:])
            ident_sb = const_p.tile([128, 128], bf16, tag="ident")
            nc.sync.dma_start(ident_sb[:], ident_d[:])
            idx_sb = const_p.tile([128, L // 16], mybir.dt.int16, tag="idx")
            nc.sync.dma_start(idx_sb[:], idx_d[:])
            dstloc_sb = const_p.tile([128, K_tot], bf16, tag="dstloc")
            nc.sync.dma_start(dstloc_sb[:], dstloc_d[:])

            # ---------------- phase A: g1 shard = d * (x_shard @ W1.T)
            scA, _ = nc.enter_named_scope("phaseA", False)
            with tc.tile_pool(name="pa_x", bufs=2) as xt_p, \
                 tc.tile_pool(name="pa_t", bufs=3) as t1_p, \
                 tc.tile_pool(name="pa_w", bufs=3) as wst_p, \
                 tc.tile_pool(name="pa_ps", bufs=2, space="PSUM") as pa, \
                 tc.tile_pool(name="pa_ps2", bufs=2, space="PSUM") as pb:
                for blki in range(NSH // BLK):
                    xts = []
                    for kc in range(KC):
                        xk = xt_p.tile([128, BLK], bf16, tag=f"xt{kc}")
                        nc.sync.dma_start(xk[:], xt_d[kc * 128:(kc + 1) * 128,
                                                      blki * BLK:(blki + 1) * BLK])
                        xts.append(xk)
                    ps_a = pa.tile([128, BLK], f32, space="PSUM", tag="psa")
                    for kc in range(KC):
                        nc.tensor.matmul(ps_a[:], lhsT=w1t_sb[kc][:], rhs=xts[kc][:],
                                         start=(kc == 0), stop=(kc == KC - 1))
                    t1t = t1_p.tile([128, BLK], bf16, tag="t1t")
                    nc.scalar.copy(t1t[:], ps_a[:])
                    sb = BLK // 128
                    ps_b = pb.tile([128, sb, 128], bf16, space="PSUM", tag="psb")
                    for s in range(sb):
                        nc.tensor.transpose(ps_b[:, s, :], t1t[:, s * 128:(s + 1) * 128],
                                            ident_sb[:])
                    wst = wst_p.tile([128, sb, HID], bf16, tag="wst")
                    # wst[p, s, f] = ps_b[p, s, f] * d[blk0 + s*128 + p]
                    nb0 = blki * sb
                    dsl = dshc_sb[:, nb0:nb0 + sb]
                    in1 = bass.AP(dsl.tensor, dsl.offset,
                                  [dsl.ap[0], [dsl.ap[1][0], sb], [0, 128]])
                    nc.vector.tensor_tensor(out=wst[:], in0=ps_b[:], in1=in1, op=OP.mult)
                    r0 = blki * BLK
                    nc.sync.dma_start(
                        g1s_d[r0:r0 + BLK, :].rearrange("(s p) f -> p s f", p=128), wst[:])
            # AllGather the shard table
            if cfg["ag_shared"]:
                nc.gpsimd.collective_compute(
                    "AllGather", OP.bypass, replica_groups=[list(range(C))],
                    ins=[g1s_d[:]], outs=[g1f_sh[:]])
                nc.sync.dma_start(g1f_d[:, :], g1f_sh[:, :])
            else:
                nc.gpsimd.collective_compute(
                    "AllGather", OP.bypass, replica_groups=[list(range(C))],
                    ins=[g1s_d[:]], outs=[g1f_d[:]])
            nc.leave_named_scope("phaseA", scA, False)
            if cfg.get("stop_after") == "A":
                _drain_out(nc, tc, outt_d)
                return

            # ---------------- message passing (both layers)
            def propagate(table_pairs, finalize):
                qrr = cfg["queue_rr"]
                with tc.tile_pool(name="mp_g", bufs=cfg["gather_bufs"]) as gath_p, \
                     tc.tile_pool(name="mp_oh", bufs=3) as oh_p, \
                     tc.tile_pool(name="mp_ps", bufs=4, space="PSUM") as psp:
                    gh = []
                    for i in range(_ceil(L, G)):
                        n_i = min(G, L - i * G)
                        gt = gath_p.tile([128, SPC, 256], bf16, tag="gt", name="gt")
                        nc.gpsimd.dma_gather(
                            out_ap=gt[:, :n_i // 128, :],
                            in_ap=table_pairs,
                            idxs_ap=idx_sb[:, i * (G // 16):i * (G // 16) + n_i // 16],
                            num_idxs=n_i,
                            num_idxs_reg=n_i,
                            elem_size=256,
                            single_packet=False,
                            queue_num=qrr[i % len(qrr)],
                        )
                        gh.append(gt)
                    kk = 0  # global chunk counter
                    kg = 0  # dstloc column counter
                    for t in range(T):
                        nch_t = gpt[t][0] + gpt[t][1]
                        ps_t = psp.tile([128, 128], f32, space="PSUM", tag="ps", name="ps_t")
                        j = 0
                        for h in (0, 1):
                            nch = gpt[t][h]
                            if nch == 0:
                                continue
                            oh = oh_p.tile([128, nch, 128], bf16, tag="oh", name="oh")
                            dsl = dstloc_sb[:, kg:kg + nch]
                            in0 = bass.AP(dsl.tensor, dsl.offset,
                                          [dsl.ap[0], [dsl.ap[1][0], nch], [0, 128]])
                            io = iota_sb[:]
                            in1 = bass.AP(io.tensor, io.offset,
                                          [io.ap[0], [0, nch], io.ap[1]])
                            nc.vector.tensor_tensor(out=oh[:], in0=in0, in1=in1,
                                                    op=OP.is_equal)
                            for jj in range(nch):
                                gt = gh[kk // SPC]
                                nc.tensor.matmul(
                                    ps_t[:],
                                    lhsT=gt[:, kk % SPC, h * 128:(h + 1) * 128],
                                    rhs=oh[:, jj, :],
                                    start=(j == 0), stop=(j == nch_t - 1))
                                kk += 1
                                j += 1
                            kg += nch
                        if nch_t == 0:
                            nc.vector.memset(ps_t[:], 0.0)
                        finalize(t, ps_t)

            with tc.tile_pool(name="ht", bufs=1) as ht_p, \
                 tc.tile_pool(name="fin", bufs=4) as fin_p:
                ht = {}

                def fin1(t, acc_t):
                    tmp = fin_p.tile([128, 128], f32, tag="tmp")
                    nc.vector.tensor_tensor(out=tmp[:], in0=acc_t[:],
                                            in1=drep_sb[:, t * 128:(t + 1) * 128],
                                            op=OP.mult)
                    h_t = ht_p.tile([128, 128], bf16, tag=f"ht{t}")
                    nc.scalar.activation(h_t[:], tmp[:], AF.Relu, bias=b1_sb[:])
                    ht[t] = h_t

                scP1, _ = nc.enter_named_scope("prop1", False)
                propagate(pair_view(g1f_d, NPAD // 2), fin1)
                nc.leave_named_scope("prop1", scP1, False)
                if cfg.get("stop_after") == "P1":
                    _drain_out(nc, tc, outt_d)
                    return

                # ---------------- phase C: g2 shard = d * (h @ Wcat.T), AllGather
                scC, _ = nc.enter_named_scope("phaseC", False)
                with tc.tile_pool(name="pc_t", bufs=4) as ct_p, \
                     tc.tile_pool(name="pc_ps", bufs=2, space="PSUM") as pc1, \
                     tc.tile_pool(name="pc_ps2", bufs=2, space="PSUM") as pc2:
                    for t in range(T):
                        ps = pc1.tile([O2, 128], f32, space="PSUM", tag="c1")
                        nc.tensor.matmul(ps[:], lhsT=wcatt_sb[:], rhs=ht[t][:],
                                         start=True, stop=True)
                        c_sb = ct_p.tile([O2, 128], bf16, tag="csb")
                        nc.scalar.copy(c_sb[:], ps[:])
                        ps2 = pc2.tile([128, O2], bf16, space="PSUM", tag="c2")
                        nc.tensor.transpose(ps2[:], c_sb[:], ident_sb[:])
                        g2t = ct_p.tile([128, O2], bf16, tag="g2t")
                        nc.vector.tensor_scalar_mul(g2t[:], ps2[:], downc_sb[:, t:t + 1])
                        nrows = min(128, NPC - t * 128)
                        nc.sync.dma_start(g2s_d[t * 128:t * 128 + nrows, :], g2t[:nrows, :])
                if cfg["ag_shared"]:
                    nc.gpsimd.collective_compute(
                        "AllGather", OP.bypass, replica_groups=[list(range(C))],
                        ins=[g2s_d[:]], outs=[g2f_sh[:]])
                    nc.sync.dma_start(g2f_d[:, :], g2f_sh[:, :])
                else:
                    nc.gpsimd.collective_compute(
                        "AllGather", OP.bypass, replica_groups=[list(range(C))],
                        ins=[g2s_d[:]], outs=[g2f_d[:]])
                nc.leave_named_scope("phaseC", scC, False)

                # ---------------- phase D: second propagate + output
                def fin2(t, acc_t):
                    tmp = fin_p.tile([128, 128], f32, tag="tmp")
                    nc.vector.tensor_tensor(out=tmp[:], in0=acc_t[:],
                                            in1=drep_sb[:, t * 128:(t + 1) * 128],
                                            op=OP.mult)
                    osb = fin_p.tile([O2, 128], f32, tag="osb")
                    nc.scalar.activation(osb[:], tmp[:], AF.Identity, bias=bcat_sb[:])
                    nc.sync.dma_start(outt_d[:, t * 128:(t + 1) * 128], osb[:])

                scP2, _ = nc.enter_named_scope("prop2", False)
                propagate(pair_view(g2f_d, N // 2), fin2)
                nc.leave_named_scope("prop2", scP2, False)


def _drain_out(nc, tc, outt_d):
    """Make truncated (stop_after) programs still produce the output tensor."""
    with tc.tile_pool(name="drain", bufs=1) as dp:
        z = dp.tile([128, 16], mybir.dt.float32, tag="z")
        nc.vector.memset(z[:], 0.0)
        nc.sync.dma_start(outt_d[:, 0:16], z[:])


def run(cfg, x, edge_index, W1, b1, W_mu, b_mu, W_logstd, b_logstd, program_cache=None,
        trace=False, result_box=None):
    meta, in_maps = preprocess(cfg, x, edge_index, W1, b1, W_mu, b_mu, W_logstd, b_logstd)
    nc = build_program(cfg, meta)
    res = run_bass_kernel_spmd(nc, in_maps, list(range(cfg["n_cores"])), trace=trace)
    if result_box is not None:
        result_box.append(res)
    N, C = cfg["n"], cfg["n_cores"]
    NPC = N // C
    O = cfg["out2"] // 2
    mu = np.empty((N, O), np.float32)
    logstd = np.empty((N, O), np.float32)
    for c in range(C):
        ot = res.results[c]["outt"]
        mu[c * NPC:(c + 1) * NPC] = ot[:O, :NPC].T
        logstd[c * NPC:(c + 1) * NPC] = ot[O:, :NPC].T
    return mu, logstd


def kernel(x, edge_index, W1, b1, W_mu, b_mu, W_logstd, b_logstd):
    mu, logstd = run(FULL_CFG, x, edge_index, W1, b1, W_mu, b_mu, W_logstd, b_logstd)
    return mu, logstd
